# revision 4
# baseline (speedup 1.0000x reference)
"""Trainium2 Bass kernel for the SRNN adapter problem.

Strategy (8 cores, data-parallel over batch B=256 -> 32 per core):
  This environment charges a large, roughly flat cost PER INSTRUCTION
  (~21us chained matmul, ~40us DVE op, ~29us DMA; engines do not overlap),
  so the kernel minimizes instruction count rather than FLOPs.

  The recurrence is computed batch-major: psum[b=32, h=1024] accumulates
      sum_dk xT_t[dk, b].T @ W_inT[dk, h-chunk]     (input projection)
    + sum_k  zT_t[k, b].T  @ (W_rec_eff - THR*I)T[k, h-chunk]
  as 2 chained groups of 14 matmuls (free dim 512, the PSUM-bank max),
  instead of the h-major form's 112 matmuls of free dim 32.
  Then v' = alpha*v + psum, z_b = v' > THR (both [32,1024], one DVE op each).
  z must return to k-major [128, 8, 32] to serve as the next step's
  stationary operand: DVE StreamTranspose (32x32 blocks) + 4 strided DMAs
  reassemble the full transpose exactly.
  u' = kappa*u + z (h-major) tracks the filtered spikes; the last 10 u's
  are snapshotted and the epilogue computes vo = W_out @ u in 8 chained
  matmuls (free dim 320 = 10 steps x 32 batch).

Host: X pre-transposed to [D, T*BL] per core; softmax+mean over the last 10
steps (0.005% of FLOPs).

All matmul arithmetic is fp32: the spiking threshold makes the system chaotic,
so bf16 would decorrelate the spike trains from the fp32 reference.
"""

import sys

sys.path.insert(0, "/opt/trn_rl_repo")

import numpy as np
from contextlib import ExitStack

from concourse import bacc, bass, mybir, tile
from concourse.bass_utils import run_bass_kernel_spmd

F32 = mybir.dt.float32
A = mybir.AluOpType

B, T, D, H, O = 256, 100, 700, 1024, 20
NCORES = 8
BL = B // NCORES  # 32 batch rows per core
KT = H // 128  # 8 k tiles
DTILES = 6  # ceil(700/128), last tile zero-padded to 128
NSTEPS = T - 1  # 99 scan steps
NTAIL = 10  # last-K softmax window
XCOLS = NSTEPS * BL  # 3168 transposed-X columns actually used

ALPHA = float(np.float32(np.exp(-1.0 / 20.0)))
KAPPA = float(np.float32(np.exp(-1.0 / 20.0)))
THR = 1.0


WI_OFF = 0
WI_LEN = DTILES * H  # 6144
W_OFF = WI_OFF + WI_LEN
W_LEN = KT * H  # 8192
WO_OFF = W_OFF + W_LEN
WO_LEN = KT * O  # 160
XT_OFF = WO_OFF + WO_LEN  # 14496
XT_LEN = DTILES * XCOLS  # 19008
BLOB_COLS = XT_OFF + XT_LEN  # 33504
XT_SPLIT = 256  # xt columns in the early DMA (covers scan steps 0..7)


def _build(nsteps=NSTEPS, nrep=1):
    nc = bacc.Bacc(None)
    blob_d = nc.declare_dram_parameter("blob", [128, BLOB_COLS], F32, isOutput=False)
    vo_d = nc.declare_dram_parameter("vo10", [O, NTAIL * BL], F32, isOutput=True)

    with ExitStack() as ctx:
        tc = ctx.enter_context(tile.TileContext(nc))
        const = ctx.enter_context(tc.tile_pool(name="const", bufs=1))
        pp = ctx.enter_context(tc.tile_pool(name="pp", bufs=1, space="PSUM"))

        blob_sb = const.tile([128, BLOB_COLS], F32, name="blob_sb")
        xt_sb = blob_sb[:, XT_OFF : XT_OFF + XT_LEN].rearrange(
            "p (a c) -> p a c", a=DTILES
        )
        wi_sb = blob_sb[:, WI_OFF : WI_OFF + WI_LEN].rearrange(
            "p (a c) -> p a c", a=DTILES
        )
        w_sb = blob_sb[:, W_OFF : W_OFF + W_LEN].rearrange("p (a c) -> p a c", a=KT)
        wo_sb = blob_sb[:, WO_OFF : WO_OFF + WO_LEN].rearrange(
            "p (a c) -> p a c", a=KT
        )
        # batch-major state on 32 partitions
        v = [const.tile([BL, H], F32, name=f"v{j}") for j in range(2)]
        zb = [const.tile([BL, H], F32, name=f"zb{j}") for j in range(2)]
        zt = [const.tile([BL, H], F32, name=f"zt{j}") for j in range(2)]
        # k-major z (stationary for the recurrence); col 33 pads the k-plane
        # stride so the transpose-DMA APs stay 3-dim
        zh = [const.tile([128, KT, 33], F32, name=f"zh{j}") for j in range(2)]
        u = [const.tile([128, KT, BL], F32, name=f"u{j}") for j in range(2)]
        usnap = const.tile([128, KT, NTAIL, BL], F32, name="usnap")
        vo_sb = const.tile([O, NTAIL * BL], F32, name="vo_sb")

        # single PSUM tensor: scan step t accumulates into banks
        # (t%2)*2 + {0,1} (rows 0:32); epilogue uses bank 4
        ps = pp.tile([128, 8, 512], F32, name="ps")

        # input DMAs, all on the sync queue (one semaphore, cumulative
        # thresholds): wi first (step 0 needs only wi + early xt), then w/wo,
        # then the bulk of xt streaming under the first scan steps
        xt_dram = blob_d[:, XT_OFF : XT_OFF + XT_LEN].rearrange(
            "p (a c) -> p a c", a=DTILES
        )
        nc.sync.dma_start(blob_sb[:, 0:W_OFF], blob_d[:, 0:W_OFF])
        nc.sync.dma_start(xt_sb[:, :, 0:XT_SPLIT], xt_dram[:, :, 0:XT_SPLIT])
        nc.sync.dma_start(blob_sb[:, W_OFF:XT_OFF], blob_d[:, W_OFF:XT_OFF])
        nc.sync.dma_start(
            xt_sb[:, :, XT_SPLIT:XCOLS], xt_dram[:, :, XT_SPLIT:XCOLS]
        )

        def u_ap(t):
            """AP holding u_t (the filtered spike train after scan step t-1)."""
            if t >= NSTEPS - NTAIL + 1:  # t >= 90: snapshot slot
                return usnap[:, :, t - (NSTEPS - NTAIL + 1), :]
            return u[t % 2][:]

        for rep in range(nrep):
            for t in range(nsteps):
                par = t % 2
                rot = t % 4
                n_mm = DTILES + (KT if t > 0 else 0)
                for c in range(2):
                    out = ps[0:BL, rot * 2 + c, :]
                    hs = slice(c * 512, (c + 1) * 512)
                    i = 0
                    # input projection first: no z dependency, so the
                    # chain's head carries only the psum-WAR wait
                    for dk in range(DTILES):
                        nc.tensor.matmul(
                            out,
                            xt_sb[:, dk, t * BL : (t + 1) * BL],
                            wi_sb[:, dk, hs],
                            start=(i == 0),
                            stop=(i == n_mm - 1),
                        )
                        i += 1
                    if t > 0:
                        for k in range(KT):
                            nc.tensor.matmul(
                                out,
                                zh[(t + 1) % 2][:, k, 0:BL],
                                w_sb[:, k, hs],
                                start=False,
                                stop=(i == n_mm - 1),
                            )
                            i += 1
                # DVE block
                if t >= 2:
                    # u_t = kappa*u_{t-1} + z_t (k-major, landed last step)
                    nc.vector.scalar_tensor_tensor(
                        u_ap(t), u_ap(t - 1), KAPPA,
                        zh[(t + 1) % 2][:, :, 0:BL], A.mult, A.add,
                    )
                elif t == 1:
                    nc.vector.tensor_copy(u_ap(1), zh[0][:, :, 0:BL])
                psum_in = ps[0:BL, rot * 2 : rot * 2 + 2, :]
                v_new = v[par][:].rearrange("b (c h) -> b c h", c=2)
                if t == 0:
                    nc.vector.tensor_copy(v_new, psum_in)
                else:
                    nc.vector.scalar_tensor_tensor(
                        v_new, v[(t + 1) % 2][:].rearrange("b (c h) -> b c h", c=2),
                        ALPHA, psum_in, A.mult, A.add,
                    )
                nc.vector.tensor_scalar(
                    zb[par][:], v[par][:], THR, None, A.is_gt
                )
                nc.vector.transpose(zt[par][:], zb[par][:])
                # reassemble full transpose: zh[32*jp+q, k, b] = z[b, 128k+32*jp+q]
                src = zt[par][:].rearrange("q (k r) -> q k r", k=KT)
                for jp in range(4):
                    nc.gpsimd.dma_start(
                        zh[par][32 * jp : 32 * jp + 32, :, 0:BL],
                        src[:, :, 32 * jp : 32 * jp + 32],
                    )

        # u_99 from the final step's z, then the output projection
        nc.vector.scalar_tensor_tensor(
            u_ap(NSTEPS), u_ap(NSTEPS - 1), KAPPA,
            zh[(nsteps - 1) % 2][:, :, 0:BL], A.mult, A.add,
        )
        vo_ps = ps[0:O, 4, 0 : NTAIL * BL]
        for k in range(KT):
            nc.tensor.matmul(
                vo_ps,
                wo_sb[:, k, :],
                usnap[:, k, :, :],
                start=(k == 0),
                stop=(k == KT - 1),
            )
        nc.vector.tensor_copy(vo_sb[:], vo_ps)
        nc.gpsimd.dma_start(vo_d[:], vo_sb[:])

    nc.compile()
    return nc


_PROGRAM = None


def _get_program():
    global _PROGRAM
    if _PROGRAM is None:
        _PROGRAM = _build()
    return _PROGRAM


def _host_prep(W_in, W_rec, W_out):
    eye = np.eye(H, dtype=np.float32)
    # z @ w_rec_eff.T - z*THR == z @ (w_rec_eff - THR*eye).T ; [k, h] layout
    WrT = (W_rec * (1.0 - eye) - np.float32(THR) * eye).T.astype(np.float32)
    WiT = np.zeros((DTILES * 128, H), np.float32)
    WiT[:D] = W_in.T.astype(np.float32)
    WoT = W_out.T.astype(np.float32)  # [H, O]
    # weight section of the blob, identical for every core: [128, cols]
    wpart = np.concatenate(
        [
            WiT.reshape(DTILES, 128, H).transpose(1, 0, 2).reshape(128, -1),
            WrT.reshape(KT, 128, H).transpose(1, 0, 2).reshape(128, -1),
            WoT.reshape(KT, 128, O).transpose(1, 0, 2).reshape(128, -1),
        ],
        axis=1,
    )
    return np.ascontiguousarray(wpart)


def kernel(X, W_in, W_rec, W_out):
    X = np.asarray(X, np.float32)
    wpart = _host_prep(
        np.asarray(W_in, np.float32), np.asarray(W_rec, np.float32),
        np.asarray(W_out, np.float32),
    )
    nc = _get_program()
    in_maps = []
    for c in range(NCORES):
        Xc = X[c * BL : (c + 1) * BL]  # [BL, T, D]
        # [D, t*BL + b] for t = 0..98 (step t uses cols t*BL:(t+1)*BL)
        XTc = np.zeros((DTILES * 128, XCOLS), np.float32)
        XTc[:D] = Xc[:, :NSTEPS, :].transpose(2, 1, 0).reshape(D, XCOLS)
        blob = np.concatenate(
            [wpart,
             XTc.reshape(DTILES, 128, XCOLS).transpose(1, 0, 2).reshape(128, -1)],
            axis=1,
        )
        in_maps.append({"blob": np.ascontiguousarray(blob)})
    res = run_bass_kernel_spmd(nc, in_maps, list(range(NCORES)))
    # vo10 per core: [O, s*BL + b] for scan steps s+89 (vo_full indices 90..99)
    vo = np.stack([r["vo10"] for r in res.results])  # [8, O, 10*BL]
    vo = vo.reshape(NCORES, O, NTAIL, BL).transpose(2, 0, 3, 1).reshape(NTAIL, B, O)
    m = vo.max(axis=2, keepdims=True)
    e = np.exp(vo - m)
    yo = e / e.sum(axis=2, keepdims=True)
    return yo.mean(axis=0).astype(np.float32)
